# revision 1
# baseline (speedup 1.0000x reference)
"""MoE layer (top-1 routing) on 8 Trainium2 NeuronCores.

Strategy: data-parallel over the batch (16 samples -> 2 per core). Routing
uses only the tiny router tables (16x8 logits), so top-1 expert selection,
the balance loss, and the per-sample expert-weight gather happen on host.
Each core runs a dense per-sample FFN (relu(x @ W1 + b1) @ W2 + b2) for its
2 samples with the gathered expert weights.

Device layout: activations are kept transposed (feature dim on SBUF
partitions) the whole way so both GEMMs contract along the partition dim
with zero on-device transposes:
    GEMM1: H^T[f,l] = sum_d W1[d,f] * X^T[d,l]   (lhsT=W1 natural layout)
    GEMM2: Y^T[m,l] = sum_f W2[f,m] * H^T[f,l]   (lhsT=W2 natural layout)
x/W1/W2 are cast to bf16 on host (PE runs bf16 at 1 row/cycle vs fp32's 4);
accumulation is fp32 in PSUM, biases applied in fp32 on the scalar engine.
"""

import numpy as np
import ml_dtypes

B, L, D, E, DFF = 16, 1024, 512, 8, 2048
N_CORES = 8
SPB = B // N_CORES  # samples per core
KD = D // 128       # contraction chunks over D
KF = DFF // 128     # contraction chunks over DFF
MD = D // 128       # output-row tiles over D
NT = L // 512       # token blocks (moving dim)

_CACHE = {}

# test.py reads exec_time_ns off this after a BASS_TRACE=1 run
LAST_RESULTS = None


def _build_nc():
    import concourse.bass as bass  # noqa: F401  (registers engines)
    import concourse.tile as tile
    from concourse import bacc, mybir

    FP = mybir.dt.float32
    BF = mybir.dt.bfloat16
    AF = mybir.ActivationFunctionType

    nc = bacc.Bacc("TRN2", target_bir_lowering=False, debug=False,
                   num_devices=N_CORES)

    xT = nc.declare_dram_parameter("xT", [SPB, D, L], BF, isOutput=False)
    w1 = nc.declare_dram_parameter("w1", [SPB, D, DFF], BF, isOutput=False)
    b1 = nc.declare_dram_parameter("b1", [SPB, 128, KF], FP, isOutput=False)
    w2 = nc.declare_dram_parameter("w2", [SPB, DFF, D], BF, isOutput=False)
    b2 = nc.declare_dram_parameter("b2", [SPB, 128, MD], FP, isOutput=False)
    outT = nc.declare_dram_parameter("outT", [SPB, D, L], FP, isOutput=True)

    with tile.TileContext(nc) as tc:
        with (
            tc.tile_pool(name="wx", bufs=2) as wx,
            tc.tile_pool(name="h", bufs=2) as hp,
            tc.tile_pool(name="o", bufs=4) as op,
            tc.tile_pool(name="ps", bufs=4, space="PSUM") as ps,
        ):
            for s in range(SPB):
                w1t = wx.tile([128, KD, DFF], BF)
                nc.sync.dma_start(
                    w1t[:], w1[s].rearrange("(k p) f -> p k f", p=128))
                xt = wx.tile([128, KD, L], BF)
                nc.sync.dma_start(
                    xt[:], xT[s].rearrange("(k p) l -> p k l", p=128))
                b1t = wx.tile([128, KF], FP)
                nc.sync.dma_start(b1t[:], b1[s])
                w2t = wx.tile([128, KF, D], BF)
                nc.sync.dma_start(
                    w2t[:], w2[s].rearrange("(k p) d -> p k d", p=128))
                b2t = wx.tile([128, MD], FP)
                nc.sync.dma_start(b2t[:], b2[s])

                # H^T, bf16, DFF on partitions (16 x [128, L])
                ht = hp.tile([128, KF, L], BF)
                for f in range(KF):
                    for t in range(NT):
                        ps1 = ps.tile([128, 512], FP)
                        for k in range(KD):
                            nc.tensor.matmul(
                                ps1[:],
                                w1t[:, k, f * 128:(f + 1) * 128],
                                xt[:, k, t * 512:(t + 1) * 512],
                                start=(k == 0), stop=(k == KD - 1))
                        nc.scalar.activation(
                            ht[:, f, t * 512:(t + 1) * 512], ps1[:],
                            AF.Relu, bias=b1t[:, f:f + 1])
                for m in range(MD):
                    for t in range(NT):
                        ps2 = ps.tile([128, 512], FP)
                        for k in range(KF):
                            nc.tensor.matmul(
                                ps2[:],
                                w2t[:, k, m * 128:(m + 1) * 128],
                                ht[:, k, t * 512:(t + 1) * 512],
                                start=(k == 0), stop=(k == KF - 1))
                        ot = op.tile([128, 512], FP)
                        nc.scalar.activation(
                            ot[:], ps2[:], AF.Identity, bias=b2t[:, m:m + 1])
                        nc.sync.dma_start(
                            outT[s, m * 128:(m + 1) * 128,
                                 t * 512:(t + 1) * 512], ot[:])
    nc.compile()
    return nc


def kernel(x, view_ids, visit_ids, router_view, router_visit, W1, b1, W2, b2):
    global LAST_RESULTS
    from concourse.bass_utils import run_bass_kernel_spmd

    x = np.asarray(x, dtype=np.float32)
    view_ids = np.asarray(view_ids)
    visit_ids = np.asarray(visit_ids)
    router_view = np.asarray(router_view, dtype=np.float32)
    router_visit = np.asarray(router_visit, dtype=np.float32)
    W1 = np.asarray(W1, dtype=np.float32)
    b1 = np.asarray(b1, dtype=np.float32)
    W2 = np.asarray(W2, dtype=np.float32)
    b2 = np.asarray(b2, dtype=np.float32)

    # --- routing + balance loss (B x E = 128 values; host) ---
    logits = router_view[view_ids] + router_visit[visit_ids]  # (B, E) f32
    top1 = logits.argmax(axis=-1)
    lmax = logits.max(axis=-1, keepdims=True)
    ex = np.exp((logits - lmax).astype(np.float64))
    probs = ex / ex.sum(axis=-1, keepdims=True)
    load = probs.mean(axis=0)
    balance_loss = np.float32(-(load * np.log(load)).sum())

    # --- host-side shard prep: gather expert weights, cast, transpose ---
    bf = ml_dtypes.bfloat16
    W1b = W1.astype(bf)
    W2b = W2.astype(bf)
    xT = np.ascontiguousarray(x.transpose(0, 2, 1)).astype(bf)  # (B, D, L)
    w1g = W1b[top1]                                   # (B, D, DFF)
    w2g = W2b[top1]                                   # (B, DFF, D)
    b1g = np.ascontiguousarray(                       # (B, 128, KF)
        b1[top1].reshape(B, KF, 128).transpose(0, 2, 1))
    b2g = np.ascontiguousarray(                       # (B, 128, MD)
        b2[top1].reshape(B, MD, 128).transpose(0, 2, 1))

    if "nc" not in _CACHE:
        _CACHE["nc"] = _build_nc()
    nc = _CACHE["nc"]

    in_maps = []
    for c in range(N_CORES):
        sl = slice(c * SPB, (c + 1) * SPB)
        in_maps.append({
            "xT": np.ascontiguousarray(xT[sl]),
            "w1": np.ascontiguousarray(w1g[sl]),
            "b1": np.ascontiguousarray(b1g[sl]),
            "w2": np.ascontiguousarray(w2g[sl]),
            "b2": np.ascontiguousarray(b2g[sl]),
        })

    res = run_bass_kernel_spmd(nc, in_maps, list(range(N_CORES)))
    LAST_RESULTS = res

    outT = np.concatenate(
        [res.results[c]["outT"] for c in range(N_CORES)], axis=0)  # (B, D, L)
    out = np.ascontiguousarray(outT.transpose(0, 2, 1))            # (B, L, D)
    return out, balance_loss


# revision 2
# speedup vs baseline: 1.0480x; 1.0480x over previous
"""MoE layer (top-1 routing) on 8 Trainium2 NeuronCores.

Strategy: data-parallel over the batch (16 samples -> 2 per core). Routing
uses only the tiny router tables (16x8 logits), so top-1 expert selection,
the balance loss, and the per-sample expert-weight gather happen on host.
Each core runs a dense per-sample FFN (relu(x @ W1 + b1) @ W2 + b2) for its
2 samples with the gathered expert weights.

Device layout: activations are kept transposed (feature dim on SBUF
partitions) the whole way so both GEMMs contract along the partition dim
with zero on-device transposes:
    GEMM1: H^T[f,l] = sum_d W1[d,f] * X^T[d,l]   (lhsT=W1 natural layout)
    GEMM2: Y^T[m,l] = sum_f W2[f,m] * H^T[f,l]   (lhsT=W2 natural layout)
x/W1/W2 are cast to bf16 on host (PE runs bf16 at 1 row/cycle vs fp32's 4);
accumulation is fp32 in PSUM, biases applied in fp32 on the scalar engine.
"""

import numpy as np
import ml_dtypes

B, L, D, E, DFF = 16, 1024, 512, 8, 2048
N_CORES = 8
SPB = B // N_CORES  # samples per core
KD = D // 128       # contraction chunks over D
KF = DFF // 128     # contraction chunks over DFF
MD = D // 128       # output-row tiles over D
NT = L // 512       # token blocks (moving dim)

_CACHE = {}

# test.py reads exec_time_ns off this after a BASS_TRACE=1 run
LAST_RESULTS = None


def _build_nc():
    import concourse.bass as bass  # noqa: F401  (registers engines)
    import concourse.tile as tile
    from concourse import bacc, mybir

    FP = mybir.dt.float32
    BF = mybir.dt.bfloat16
    AF = mybir.ActivationFunctionType

    nc = bacc.Bacc("TRN2", target_bir_lowering=False, debug=False,
                   num_devices=N_CORES)

    xT = nc.declare_dram_parameter("xT", [SPB, D, L], BF, isOutput=False)
    w1 = nc.declare_dram_parameter("w1", [SPB, D, DFF], BF, isOutput=False)
    b1 = nc.declare_dram_parameter("b1", [SPB, 128, KF], FP, isOutput=False)
    w2 = nc.declare_dram_parameter("w2", [SPB, DFF, D], BF, isOutput=False)
    b2 = nc.declare_dram_parameter("b2", [SPB, 128, MD], FP, isOutput=False)
    outT = nc.declare_dram_parameter("outT", [SPB, D, L], FP, isOutput=True)

    HF = DFF // 2  # f-column half of W1 (prefetch granule)

    with tile.TileContext(nc) as tc:
        with (
            tc.tile_pool(name="wx", bufs=2) as wx,
            tc.tile_pool(name="h", bufs=2) as hp,
            tc.tile_pool(name="o", bufs=4) as op,
            tc.tile_pool(name="ps", bufs=4, space="PSUM") as ps,
        ):
            for s in range(SPB):
                w1d = w1[s].rearrange("(k p) f -> p k f", p=128)
                xd = xT[s].rearrange("(k p) l -> p k l", p=128)
                w2d = w2[s].rearrange("(k p) d -> p k d", p=128)

                # Loads split into 8-queue-sized granules, issued in order
                # of first use so the first matmul group is gated on ~2 MiB
                # (w1 f-half 0 + x token-block 0), not the whole 6 MiB.
                w1t = wx.tile([128, KD, DFF], BF)
                xt = wx.tile([128, KD, L], BF)
                b1t = wx.tile([128, KF], FP)
                for k in range(KD):
                    nc.sync.dma_start(w1t[:, k, 0:HF], w1d[:, k, 0:HF])
                for k in range(KD):
                    nc.sync.dma_start(xt[:, k, 0:512], xd[:, k, 0:512])
                nc.sync.dma_start(b1t[:], b1[s])
                for k in range(KD):
                    nc.sync.dma_start(xt[:, k, 512:L], xd[:, k, 512:L])
                for k in range(KD):
                    nc.sync.dma_start(w1t[:, k, HF:DFF], w1d[:, k, HF:DFF])

                # H^T, bf16, DFF on partitions (16 x [128, L])
                ht = hp.tile([128, KF, L], BF)
                for fh in range(2):
                    for t in range(NT):
                        for f in range(fh * KF // 2, (fh + 1) * KF // 2):
                            ps1 = ps.tile([128, 512], FP)
                            for k in range(KD):
                                nc.tensor.matmul(
                                    ps1[:],
                                    w1t[:, k, f * 128:(f + 1) * 128],
                                    xt[:, k, t * 512:(t + 1) * 512],
                                    start=(k == 0), stop=(k == KD - 1))
                            nc.scalar.activation(
                                ht[:, f, t * 512:(t + 1) * 512], ps1[:],
                                AF.Relu, bias=b1t[:, f:f + 1])

                w2t = wx.tile([128, KF, D], BF)
                b2t = wx.tile([128, MD], FP)
                for k in range(KF):
                    nc.sync.dma_start(w2t[:, k, :], w2d[:, k, :])
                nc.sync.dma_start(b2t[:], b2[s])

                for m in range(MD):
                    for t in range(NT):
                        ps2 = ps.tile([128, 512], FP)
                        for k in range(KF):
                            nc.tensor.matmul(
                                ps2[:],
                                w2t[:, k, m * 128:(m + 1) * 128],
                                ht[:, k, t * 512:(t + 1) * 512],
                                start=(k == 0), stop=(k == KF - 1))
                        ot = op.tile([128, 512], FP)
                        nc.scalar.activation(
                            ot[:], ps2[:], AF.Identity, bias=b2t[:, m:m + 1])
                        nc.sync.dma_start(
                            outT[s, m * 128:(m + 1) * 128,
                                 t * 512:(t + 1) * 512], ot[:])
    nc.compile()
    return nc


def kernel(x, view_ids, visit_ids, router_view, router_visit, W1, b1, W2, b2):
    global LAST_RESULTS
    from concourse.bass_utils import run_bass_kernel_spmd

    x = np.asarray(x, dtype=np.float32)
    view_ids = np.asarray(view_ids)
    visit_ids = np.asarray(visit_ids)
    router_view = np.asarray(router_view, dtype=np.float32)
    router_visit = np.asarray(router_visit, dtype=np.float32)
    W1 = np.asarray(W1, dtype=np.float32)
    b1 = np.asarray(b1, dtype=np.float32)
    W2 = np.asarray(W2, dtype=np.float32)
    b2 = np.asarray(b2, dtype=np.float32)

    # --- routing + balance loss (B x E = 128 values; host) ---
    logits = router_view[view_ids] + router_visit[visit_ids]  # (B, E) f32
    top1 = logits.argmax(axis=-1)
    lmax = logits.max(axis=-1, keepdims=True)
    ex = np.exp((logits - lmax).astype(np.float64))
    probs = ex / ex.sum(axis=-1, keepdims=True)
    load = probs.mean(axis=0)
    balance_loss = np.float32(-(load * np.log(load)).sum())

    # --- host-side shard prep: gather expert weights, cast, transpose ---
    bf = ml_dtypes.bfloat16
    W1b = W1.astype(bf)
    W2b = W2.astype(bf)
    xT = np.ascontiguousarray(x.transpose(0, 2, 1)).astype(bf)  # (B, D, L)
    w1g = W1b[top1]                                   # (B, D, DFF)
    w2g = W2b[top1]                                   # (B, DFF, D)
    b1g = np.ascontiguousarray(                       # (B, 128, KF)
        b1[top1].reshape(B, KF, 128).transpose(0, 2, 1))
    b2g = np.ascontiguousarray(                       # (B, 128, MD)
        b2[top1].reshape(B, MD, 128).transpose(0, 2, 1))

    if "nc" not in _CACHE:
        _CACHE["nc"] = _build_nc()
    nc = _CACHE["nc"]

    in_maps = []
    for c in range(N_CORES):
        sl = slice(c * SPB, (c + 1) * SPB)
        in_maps.append({
            "xT": np.ascontiguousarray(xT[sl]),
            "w1": np.ascontiguousarray(w1g[sl]),
            "b1": np.ascontiguousarray(b1g[sl]),
            "w2": np.ascontiguousarray(w2g[sl]),
            "b2": np.ascontiguousarray(b2g[sl]),
        })

    res = run_bass_kernel_spmd(nc, in_maps, list(range(N_CORES)))
    LAST_RESULTS = res

    outT = np.concatenate(
        [res.results[c]["outT"] for c in range(N_CORES)], axis=0)  # (B, D, L)
    out = np.ascontiguousarray(outT.transpose(0, 2, 1))            # (B, L, D)
    return out, balance_loss
